# revision 14
# baseline (speedup 1.0000x reference)
"""Causal GQA self-attention (B=2,S=2048,H=2048,NH=16,NKV=4,HD=128) on 8 TRN2 cores.

Sharding: 2-way data-parallel over batch x 4-way tensor-parallel over heads.
Core c = 4*b + t handles batch b, q-heads 4t..4t+3, kv-head t (GQA group t).

Per-core pipeline (fp32 storage, fp32r tensor-engine compute):
  - K/V projected in natural [keys, HD] layout (k-norm + rope cheap there),
    normalized k transposed to [HD, keys] via PE transpose.
  - Q projected directly transposed [HD, seq]; rms-norm via ones-matmul
    partition reduction; rotate-half via a signed permutation matmul.
  - scores computed transposed [keys, queries] so softmax'd probs land in the
    exact layout the PV matmul needs (no per-tile transposes); softmax without
    max-subtraction (rms-normed q,k bound |score| <= sqrt(128)); denominator
    via an all-ones matmul that also yields it pre-broadcast.
  - Head-sharded attention outputs redistributed with one 8-core AllToAll
    (256-query blocks), then o_proj is a full contraction per query block, so
    the host only concatenates disjoint output slices.

`unroll` builds N chained copies of the full pipeline in one NEFF (used by
test.py to measure per-iteration HW time as a wall-clock slope).
"""
import sys
import os

for _p in ("/opt/trn_rl_repo", "/root/.axon_site/_ro/trn_rl_repo"):
    if os.path.isdir(_p) and _p not in sys.path:
        sys.path.insert(0, _p)

import numpy as np
import ml_dtypes
import concourse.bass as bass
import concourse.tile as tile
from concourse import bacc, mybir
from concourse.bass_utils import run_bass_kernel_spmd

B, S, H = 2, 2048, 2048
NH, NKV, HD = 16, 4, 128
EPS = 1e-6
P = 128
F32 = mybir.dt.float32
F32R = mybir.dt.float32r
BF16 = mybir.dt.bfloat16
AF = mybir.ActivationFunctionType
ALU = mybir.AluOpType

_NC_CACHE = {}


def _emit_oproj(nc, tc, d, a2a_src):
    with (
        tc.tile_pool(name="yta", bufs=1) as ytp,
        tc.tile_pool(name="wos", bufs=3) as wsp,
        tc.tile_pool(name="osb", bufs=3) as osp,
        tc.tile_pool(name="psO", bufs=4, space="PSUM") as psO,
    ):
        yta = ytp.tile([P, 32, 256], BF16, tag="yta")
        for bp in range(2):
            for hc in range(16):
                row = 2048 * bp + 128 * hc
                nc.sync.dma_start(yta[:, 16 * bp + hc, :],
                                  a2a_src[row:row + 128, :])
        for oc in range(4):
            wh = []
            for hf in range(2):
                t = wsp.tile([P, 8, 512], BF16, tag="wos")
                # chunk the load so hc=0 lands early
                for q4 in range(4):
                    nc.sync.dma_start(
                        t[:, 2 * q4:2 * (q4 + 1), :],
                        d["wo"].ap()[1024 * hf + 256 * q4:1024 * hf + 256 * (q4 + 1),
                                     512 * oc:512 * (oc + 1)]
                        .rearrange("(c p) s -> p c s", p=P))
                wh.append(t)

            def wob(hc, wh=wh):
                return wh[hc // 8][:, hc % 8, :]
            for bp in range(2):
                for qt in range(2):
                    o_ps = psO.tile([P, 512], F32, tag="oacc")
                    for hc in range(16):
                        nc.tensor.matmul(
                            o_ps[:],
                            yta[:, 16 * bp + hc, 128 * qt:128 * (qt + 1)],
                            wob(hc),
                            start=(hc == 0), stop=(hc == 15))
                    osb = osp.tile([P, 512], F32, tag="osb")
                    nc.vector.tensor_copy(osb[:], o_ps[:])
                    nc.sync.dma_start(
                        d["o_out"].ap()[bp, 128 * qt:128 * (qt + 1),
                                        512 * oc:512 * (oc + 1)],
                        osb[:])


def _emit_iteration(nc, tc, d, a2a_in, a2a_out, skip_collective, C):
    r32 = lambda ap: ap.bitcast(F32R)
    with (
        tc.tile_pool(name="consts", bufs=1) as cp,
        tc.tile_pool(name="stream", bufs=3) as sp,
        tc.tile_pool(name="q2p", bufs=2) as q2p,
        tc.tile_pool(name="t1p", bufs=2) as t1p,
        tc.tile_pool(name="t2p", bufs=2) as t2p,
        tc.tile_pool(name="q12p", bufs=2) as q12p,
        tc.tile_pool(name="qsb", bufs=2) as qsp,
        tc.tile_pool(name="sqb", bufs=3) as sqp,
        tc.tile_pool(name="qtn", bufs=5) as qnp,
        tc.tile_pool(name="pt", bufs=4) as ptp,
        tc.tile_pool(name="fout", bufs=3) as fop,
        tc.tile_pool(name="ktmp", bufs=3) as kp,
        tc.tile_pool(name="psA", bufs=3, space="PSUM") as psA,
        tc.tile_pool(name="psAcc", bufs=1, space="PSUM") as psAcc,
        tc.tile_pool(name="psKv", bufs=2, space="PSUM") as psKv,
    ):
        (wkv_t, wq_t, mrot_t, ones_t, ident_t, masks_t, epsk_t, epsq_t,
         deferred) = C
        kT_all = cp.tile([P, 16, HD], F32R, tag="kT")
        v_all = cp.tile([P, 16, HD], F32R, tag="v")
        xh0 = []
        for hf in range(2):
            t = sp.tile([P, 8, 512], F32R, tag="stream")
            nc.sync.dma_start(
                t[:],
                r32(d["xT"].ap()[1024 * hf:1024 * (hf + 1), 0:512]
                    .rearrange("(c p) s -> p c s", p=P)))
            xh0.append(t)
        if deferred is not None:
            deferred()

        # ---- main pass over 512-column blocks ----
        for jq in range(4):
            if jq == 0:
                xh = xh0
            else:
                xh = []
                for hf in range(2):
                    t = sp.tile([P, 8, 512], F32R, tag="stream")
                    nc.sync.dma_start(
                        t[:],
                        r32(d["xT"].ap()[1024 * hf:1024 * (hf + 1),
                                         512 * jq:512 * (jq + 1)]
                            .rearrange("(c p) s -> p c s", p=P)))
                    xh.append(t)

            def xblk(hc):
                return xh[hc // 8][:, hc % 8, :]

            cosk_t = sp.tile([P, 4, HD], F32, tag="cosks")
            nc.sync.dma_start(
                cosk_t[:], d["cosk"].ap()[512 * jq:512 * (jq + 1), :]
                .rearrange("(c p) n -> p c n", p=P))
            sink_t = sp.tile([P, 4, HD], F32, tag="sinks")
            nc.sync.dma_start(
                sink_t[:], d["sink"].ap()[512 * jq:512 * (jq + 1), :]
                .rearrange("(c p) n -> p c n", p=P))
            cosq_t = sp.tile([P, 512], F32, tag="cosqs")
            nc.sync.dma_start(cosq_t[:], d["cosq"].ap()[:, 512 * jq:512 * (jq + 1)])
            sinq_t = sp.tile([P, 512], F32, tag="sinqs")
            nc.sync.dma_start(sinq_t[:], d["sinq"].ap()[:, 512 * jq:512 * (jq + 1)])

            # -- KV projection + k norm/rope for key tiles 4jq..4jq+3 --
            for r in range(4):
                kt_i = 4 * jq + r
                kv_ps = psKv.tile([P, 256], F32, tag="kv")
                for hc in range(16):
                    nc.tensor.matmul(kv_ps[:], xblk(hc)[:, 128 * r:128 * (r + 1)],
                                     wkv_t[:, hc, :],
                                     start=(hc == 0), stop=(hc == 15))
                ksb = kp.tile([P, HD], F32, tag="ksb")
                nc.vector.tensor_copy(ksb[:], kv_ps[:, 0:HD])
                nc.vector.tensor_copy(v_all[:, kt_i, :], kv_ps[:, HD:256])
                kscr = kp.tile([P, HD], F32, tag="kscr")
                ks2 = kp.tile([P, 1], F32, tag="ks2")
                nc.vector.scalar_tensor_tensor(
                    out=kscr[:], in0=ksb[:], scalar=1.0,
                    in1=ksb[:], op0=ALU.mult, op1=ALU.mult,
                    accum_out=ks2[:])
                lnk = kp.tile([P, 1], F32, tag="lnk")
                nc.scalar.activation(lnk[:], ks2[:], AF.Ln,
                                     bias=epsk_t[:], scale=1.0 / HD)
                rk = kp.tile([P, 1], F32, tag="rk")
                nc.scalar.activation(rk[:], lnk[:], AF.Exp, scale=-0.5)
                t1k = kp.tile([P, HD], F32, tag="t1k")
                nc.vector.tensor_tensor(out=t1k[:], in0=ksb[:],
                                        in1=cosk_t[:, r, :], op=ALU.mult)
                t2k = kp.tile([P, HD], F32, tag="t2k")
                wrap = bass.AP(ksb.tensor, ksb.offset + 64,
                               [list(ksb.ap[0]), [-64, 2], [1, 64]])
                nc.vector.tensor_tensor(
                    out=t2k[:].rearrange("p (a b) -> p a b", a=2),
                    in0=wrap,
                    in1=sink_t[:, r, :].rearrange("p (a b) -> p a b", a=2),
                    op=ALU.mult)
                k12 = kp.tile([P, HD], F32, tag="k12")
                nc.vector.tensor_tensor(out=k12[:], in0=t1k[:], in1=t2k[:],
                                        op=ALU.add)
                khat = kp.tile([P, HD], F32, tag="khat")
                nc.vector.tensor_scalar_mul(khat[:], k12[:], rk[:])
                ktr_full = psKv.tile([P, 256], F32, tag="kv")
                ktr = ktr_full[:, 0:HD]
                nc.tensor.transpose(ktr[:], khat[:], ident_t[:])
                nc.vector.tensor_copy(kT_all[:, kt_i, :], ktr[:])

            # -- Q proj + norm + rope for all 4 heads (ACT does sqrts here) --
            qT_n = {}
            for h in range(4):
                q_ps = psA.tile([P, 512], F32, tag="big")
                for hc in range(16):
                    nc.tensor.matmul(q_ps[:], wq_t[:, hc, 128 * h:128 * (h + 1)],
                                     xblk(hc),
                                     start=(hc == 0), stop=(hc == 15))
                qsb = qsp.tile([P, 512], F32R, tag="qsb")
                nc.vector.tensor_copy(qsb[:], q_ps[:])
                q2 = q2p.tile([P, 512], F32R, tag="q2")
                nc.vector.tensor_tensor(out=q2[:], in0=qsb[:], in1=qsb[:],
                                        op=ALU.mult)
                ssum_ps = psA.tile([P, 512], F32, tag="big")
                nc.tensor.matmul(ssum_ps[:], ones_t[:], q2[:],
                                 start=True, stop=True)
                lnB = sqp.tile([P, 512], F32, tag="sqb")
                nc.scalar.activation(lnB[:], ssum_ps[:], AF.Ln,
                                     bias=epsq_t[:], scale=1.0)
                rqB = sqp.tile([P, 512], F32, tag="sqb")
                nc.scalar.activation(rqB[:], lnB[:], AF.Exp, scale=-0.5)
                rot_ps = psA.tile([P, 512], F32, tag="big")
                nc.tensor.matmul(rot_ps[:], mrot_t[:], qsb[:],
                                 start=True, stop=True)
                t1 = t1p.tile([P, 512], F32, tag="t1")
                nc.gpsimd.tensor_tensor(
                    out=t1[:], in0=qsb[:],
                    in1=cosq_t[:], op=ALU.mult)
                t2 = t2p.tile([P, 512], F32, tag="t2")
                nc.vector.tensor_tensor(
                    out=t2[:], in0=rot_ps[:],
                    in1=sinq_t[:], op=ALU.mult)
                q12 = q12p.tile([P, 512], F32, tag="q12")
                nc.vector.tensor_tensor(out=q12[:], in0=t1[:], in1=t2[:],
                                        op=ALU.add)
                qt = qnp.tile([P, 512], F32R, tag="qtn")
                nc.vector.tensor_tensor(out=qt[:], in0=q12[:], in1=rqB[:],
                                        op=ALU.mult)
                qT_n[h] = qt

            # -- attention for all 4 heads (ACT does exps here) --
            for h in range(4):
                nch = 4 * jq + 4
                y_ps = psAcc.tile([P, 512], F32, tag="yacc")
                d_ps = psAcc.tile([P, 512], F32, tag="dacc")
                for ci in range(nch):
                    r = ci - 4 * jq
                    # diagonal chunks: restrict to the allowed query range
                    off = 0 if r < 1 else (128 if r == 1 else 256)
                    s_ps = psA.tile([P, 512], F32, tag="big")
                    nc.tensor.matmul(s_ps[:, off:512], kT_all[:, ci, :],
                                     qT_n[h][:, off:512],
                                     start=True, stop=True)
                    pt = ptp.tile([P, 512], F32R, tag="pt")
                    nc.scalar.activation(pt[:, off:512], s_ps[:, off:512], AF.Exp)
                    if r >= 0:
                        moff = (0, 512, 896, 1152)[r]
                        nc.gpsimd.tensor_tensor(
                            out=pt[:, off:512], in0=pt[:, off:512],
                            in1=masks_t[:, moff:moff + (512 - off)], op=ALU.mult)
                    nc.tensor.matmul(y_ps[:, off:512], v_all[:, ci, :],
                                     pt[:, off:512],
                                     start=(ci == 0), stop=(ci == nch - 1))
                    nc.tensor.matmul(d_ps[:, off:512], ones_t[:],
                                     pt[:, off:512],
                                     start=(ci == 0), stop=(ci == nch - 1))
                rden = sqp.tile([P, 512], F32, tag="sqb")
                nc.vector.reciprocal(rden[:], d_ps[:])
                yh = fop.tile([P, 512], BF16, tag="fout")
                nc.vector.tensor_tensor(out=yh[:], in0=y_ps[:], in1=rden[:],
                                        op=ALU.mult)
                for half in range(2):
                    j = 2 * jq + half
                    nc.sync.dma_start(
                        a2a_in[512 * j + 128 * h:512 * j + 128 * (h + 1), :],
                        yh[:, 256 * half:256 * (half + 1)])

        # ---- redistribute: 8-core AllToAll ----
        if not skip_collective:
            nc.gpsimd.collective_compute(
                "AllToAll", ALU.bypass,
                replica_groups=[[0, 1, 2, 3, 4, 5, 6, 7]],
                ins=[a2a_in.opt()],
                outs=[a2a_out.opt()])

    a2a_src = a2a_in if skip_collective else a2a_out
    _emit_oproj(nc, tc, d, a2a_src)


def _build_nc(unroll=1, skip_collective=False):
    nc = bacc.Bacc("TRN2", target_bir_lowering=False, debug=False, num_devices=8)

    d = {}
    for name, shape in [
        ("xT", [H, S]), ("wq", [H, 512]), ("wkv", [H, 256]),
        ("cosq", [HD, S]), ("sinq", [HD, S]), ("cosk", [S, HD]),
        ("sink", [S, HD]), ("mrot", [HD, HD]), ("masks", [P, 1408]),
        ("onesm", [P, P]), ("ident", [P, P]),
    ]:
        d[name] = nc.dram_tensor(name, shape, F32, kind="ExternalInput")
    d["wo"] = nc.dram_tensor("wo", [H, H], BF16, kind="ExternalInput")
    d["o_out"] = nc.dram_tensor("o_out", [2, 256, H], F32, kind="ExternalOutput")

    r32 = lambda ap: ap.bitcast(F32R)
    with tile.TileContext(nc) as tc:
        with (
            tc.tile_pool(name="dram", bufs=1, space="DRAM") as dram,
            tc.tile_pool(name="gconsts", bufs=1) as gp,
        ):
            a2a_in = dram.tile([8 * 512, 256], BF16, tag="a2a_in")
            a2a_out = dram.tile([8 * 512, 256], BF16, tag="a2a_out")
            wkv_t = gp.tile([P, 16, 256], F32R, tag="wkv")
            for q4 in range(4):
                nc.sync.dma_start(
                    wkv_t[:, 4 * q4:4 * (q4 + 1), :],
                    r32(d["wkv"].ap()[512 * q4:512 * (q4 + 1), :]
                        .rearrange("(c p) n -> p c n", p=P)))
            wq_t = gp.tile([P, 16, 512], F32R, tag="wq")
            mrot_t = gp.tile([P, P], F32R, tag="mrot")
            nc.sync.dma_start(mrot_t[:], r32(d["mrot"].ap()))
            ones_t = gp.tile([P, P], F32R, tag="ones")
            nc.sync.dma_start(ones_t[:], r32(d["onesm"].ap()))
            ident_t = gp.tile([P, P], F32, tag="ident")
            nc.sync.dma_start(ident_t[:], d["ident"].ap())
            masks_t = gp.tile([P, 1408], F32R, tag="masks")

            def _deferred():
                for q4 in range(4):
                    nc.sync.dma_start(
                        wq_t[:, 4 * q4:4 * (q4 + 1), :],
                        r32(d["wq"].ap()[512 * q4:512 * (q4 + 1), :]
                            .rearrange("(c p) n -> p c n", p=P)))
                nc.sync.dma_start(masks_t[:], r32(d["masks"].ap()))
            epsk_t = gp.tile([P, 1], F32, tag="epsk")
            nc.vector.memset(epsk_t[:], EPS)
            epsq_t = gp.tile([P, 1], F32, tag="epsq")
            nc.vector.memset(epsq_t[:], HD * EPS)
            for it in range(unroll):
                C = (wkv_t, wq_t, mrot_t, ones_t, ident_t, masks_t, epsk_t,
                     epsq_t, _deferred if it == 0 else None)
                _emit_iteration(nc, tc, d, a2a_in, a2a_out, skip_collective, C)

    # Force Exp and Ln onto the shared 'natural_log_exp_and_others' ACT
    # table set: hide exp/ln from every other set during the act-table pass
    # (strict subsets, so the chosen set always really contains the func).
    import concourse.bacc as _bacc_mod
    import concourse.hw_specs as _hws
    _orig_tables = _bacc_mod.get_activation_tables

    def _patched_tables(arch):
        t = dict(_orig_tables(arch))
        for name in t:
            if name != "natural_log_exp_and_others":
                t[name] = t[name] - {AF.Exp, AF.Ln}
        return t

    _bacc_mod.get_activation_tables = _patched_tables
    try:
        nc.compile()
    finally:
        _bacc_mod.get_activation_tables = _orig_tables
    return nc


def _host_prep(x, rotary_cos, rotary_sin, Wq, Wk, Wv, Wo, q_norm_w, k_norm_w):
    """Shard + re-lay-out inputs for the 8 cores. Pure marshalling + table
    baking (no reductions)."""
    x = np.ascontiguousarray(np.asarray(x, dtype=np.float32))
    cos = np.asarray(rotary_cos, dtype=np.float32)
    sin = np.asarray(rotary_sin, dtype=np.float32)
    Wq = np.asarray(Wq, dtype=np.float32)
    Wk = np.asarray(Wk, dtype=np.float32)
    Wv = np.asarray(Wv, dtype=np.float32)
    Wo = np.ascontiguousarray(np.asarray(Wo, dtype=np.float32).astype(ml_dtypes.bfloat16))
    qw = np.asarray(q_norm_w, dtype=np.float32)
    kw = np.asarray(k_norm_w, dtype=np.float32)

    rot_idx = (np.arange(HD) + 64) % HD
    cosq = np.ascontiguousarray((cos * qw[None, :]).T)
    sinq = np.ascontiguousarray((sin * qw[rot_idx][None, :]).T)
    Rm = np.zeros((HD, HD), dtype=np.float32)
    for dd in range(64):
        Rm[dd, dd + 64] = -1.0
        Rm[dd + 64, dd] = 1.0
    mrot = np.ascontiguousarray(Rm.T)
    cosk = np.ascontiguousarray(cos * kw[None, :])
    sink = np.ascontiguousarray(np.concatenate(
        [-sin[:, :64] * kw[None, 64:], sin[:, 64:] * kw[None, :64]], axis=1))
    kk = np.arange(P)[:, None]
    qq = np.arange(512)[None, :]
    m = [((128 * r + kk) <= qq).astype(np.float32) for r in range(4)]
    masks = np.ascontiguousarray(np.concatenate(
        [m[0], m[1][:, 128:], m[2][:, 256:], m[3][:, 256:]], axis=1))
    onesm = np.ones((P, P), dtype=np.float32)
    ident = np.eye(P, dtype=np.float32)

    xT = [np.ascontiguousarray(x[b].T) for b in range(B)]
    wq_s = [np.ascontiguousarray(Wq[:, t * 512:(t + 1) * 512]) for t in range(4)]
    wkv_s = [np.ascontiguousarray(np.concatenate(
        [Wk[:, t * HD:(t + 1) * HD], Wv[:, t * HD:(t + 1) * HD]], axis=1))
        for t in range(4)]

    in_maps = []
    for c in range(8):
        b, t = c // 4, c % 4
        in_maps.append({
            "xT": xT[b], "wq": wq_s[t], "wkv": wkv_s[t], "wo": Wo,
            "cosq": cosq, "sinq": sinq, "cosk": cosk, "sink": sink,
            "mrot": mrot, "masks": masks, "onesm": onesm, "ident": ident,
        })
    return in_maps


def kernel(**inputs):
    if "nc" not in _NC_CACHE:
        _NC_CACHE["nc"] = _build_nc()
    nc = _NC_CACHE["nc"]
    in_maps = _host_prep(**inputs)
    res = run_bass_kernel_spmd(nc, in_maps, list(range(8))).results
    out = np.empty((B, S, H), dtype=np.float32)
    for j in range(8):
        o = res[j]["o_out"]
        for b in range(B):
            out[b, 256 * j:256 * (j + 1), :] = o[b]
    return out
